# revision 64
# baseline (speedup 1.0000x reference)
"""3-layer GCN encoder (PyG GCNConv semantics) on 8 Trainium2 NeuronCores.

Strategy (dst-sharded message passing, gather-descriptor-rate bound):
  - Nodes are 1D-partitioned across the 8 cores (node n -> core n // (N/8)).
  - Per layer l:  z = (dinv * x) @ W_l   computed shard-wise in fp32 on PE,
    rounded to bf16, AllGathered into a full [N, D] bf16 node-order table in
    DRAM. The layer-0 table is precomputed on the HOST (plain numpy), so the
    device starts gathering immediately. (norm factorizes: out[i] = dinv[i]
    * sum_e dinv[src_e] * (x@W)[src_e], so no per-edge scaling is needed.)
  - Each core owns the edges whose dst lands in its shard. Edge messages are
    fetched with GPSIMD dma_gather (HBM->SBUF, 256B rows) through a SINGLE
    base view at row 32768 with SIGNED int16 idx = node - 32768 (the SWDGE
    address mul-acc is unsigned*signed; verified on HW). Only trailing
    negative idxs of a chunk are dropped, so the host keeps the last token
    of every chunk non-negative.
  - Aggregation on PE: per 128-token slot, a HOST-precomputed one-hot tile
    H[t, dstlocal_t] (bf16, streamed per chunk via HWDGE) is matmul'ed with
    the gathered tokens, accumulating fp32 PSUM per 128-node dst block; the
    self-loop term is folded in as one extra `identity @ z_own` matmul.
    Blocks are packed in PAIRS sharing a slot range (halves slot-padding);
    the per-(slot,block) tile lists are core-invariant (union over cores).
  - Epilogue: e = dinv * (agg + z_own) via ONE ScalarE activation (per-
    partition scale); nothing on the Vector engine touches the critical
    path (DVE ops crawl 25-150x while gathers run; ACT is immune).

Host side: edge sort by (core, block), pair-dense packing with (idx=0,
dstl=-1) pad dummies (H row all-zero), H/idx array packing, layer-0 table
compute, and final unshard (concat + slice).
"""

import math

import numpy as np
import ml_dtypes

from concourse import bass, bacc, mybir, library_config
import concourse.tile as tile

BF16 = ml_dtypes.bfloat16
P = 128
LOW_LIM = 32768
F32 = mybir.dt.float32
BF = mybir.dt.bfloat16
I16 = mybir.dt.int16


# ----------------------------------------------------------------------------
# host-side preprocessing
# ----------------------------------------------------------------------------

class Plan:
    """Static (core-invariant) program structure + per-core packed arrays."""
    pass


def build_plan(edge_index, n, n_cores, group_blocks=3, max_chunk_slots=28):
    src_e = np.asarray(edge_index[0], dtype=np.int64)
    dst_e = np.asarray(edge_index[1], dtype=np.int64)
    # self-loops are handled analytically on-device (identity matmul), so the
    # token stream only carries the real edges; degree still counts them.
    src = src_e
    dst = dst_e

    deg = (np.bincount(dst, minlength=n) + 1).astype(np.float64)
    dinv = (1.0 / np.sqrt(deg)).astype(np.float32)

    assert n % n_cores == 0
    npc = n // n_cores
    nb = math.ceil(npc / P)
    npc_pad = nb * P

    # z-table is in plain node order; gathers use ONE base view at row
    # LOW_LIM with SIGNED int16 indices idx = node - LOW_LIM (the SWDGE
    # mul-acc is unsigned*signed, so negative idx reads below the view).
    assert n - LOW_LIM <= LOW_LIM

    # table order: [all cores' prefix blocks 0..nbA-1 | all cores' suffix]
    # so the big prefix AllGather can fire early (after block nbA-1's z).
    nbA = nb - 9
    hA = nbA * P
    hB = npc - hA
    s_core = src // npc
    s_loc = src % npc
    tpos = np.where(
        s_loc < hA,
        s_core * hA + s_loc,
        n_cores * hA + s_core * hB + (s_loc - hA),
    )

    core = dst // npc
    blk = (dst % npc) // P
    dstl = (dst % npc) % P
    gidx_val = tpos - LOW_LIM

    # stable-sort edges by (core, blk) -> contiguous per-block groups
    key = core * nb + blk
    order = np.argsort(key, kind="stable")
    key_s = key[order]
    gidx_s = gidx_val[order]
    dstl_s = dstl[order]

    ngroups_keys = n_cores * nb
    cnt = np.bincount(key_s, minlength=ngroups_keys)
    cnt_b = cnt.reshape(n_cores, nb)

    # FULL-DENSE packing: one dense token stream per core; block r's core-c
    # tokens occupy positions [cum[c,r], cum[c,r]+cnt_b[c,r]). The program-
    # level (core-invariant) slot coverage of a block is the union over
    # cores; PSUM accumulation chains may span chunk boundaries.
    cum = np.zeros((n_cores, nb + 1), dtype=np.int64)
    np.cumsum(cnt_b, axis=1, out=cum[:, 1:])
    S_total = int(math.ceil(cum[:, nb].max() / P))
    assert (cnt_b > 0).all()
    start_sl = (cum[:, :nb] // P).min(axis=0)
    end_sl = ((cum[:, :nb] + cnt_b - 1) // P).max(axis=0)
    end_sl = np.maximum(end_sl, start_sl)
    assert (np.diff(start_sl) >= 0).all() and (np.diff(end_sl) >= 0).all()
    # blocks covering slot s form the contiguous range [lo_b(s), hi_b(s)]
    sarange = np.arange(S_total)
    lo_b = np.searchsorted(end_sl, sarange, side="left")
    hi_b = np.minimum(np.searchsorted(start_sl, sarange, side="right") - 1,
                      nb - 1)
    assert (lo_b <= hi_b).all()
    ntile_s = hi_b - lo_b + 1
    base_t = np.zeros(S_total + 1, dtype=np.int64)
    np.cumsum(ntile_s, out=base_t[1:])
    NT = int(base_t[S_total])

    # chunks: plain slot ranges; ents carry first/last flags for chain
    # start/stop across chunk boundaries
    # chunk boundaries; the FINAL chunk is kept small so the post-gather
    # aggregation tail (which gates the AllGather / kernel end) is short
    bounds = []
    c0 = 0
    tail_sl = min(24, S_total)
    while c0 < S_total - tail_sl:
        ns = min(max_chunk_slots, S_total - tail_sl - c0)
        bounds.append((c0, ns))
        c0 += ns
    while c0 < S_total:
        ns = min(12, S_total - c0)
        bounds.append((c0, ns))
        c0 += ns

    chunks = []  # (slot0, ns, tile0, ntiles, [(r, slots, tiles, fst, lst)])
    for (c0, ns) in bounds:
        ents = []
        for r in range(int(lo_b[c0]), int(hi_b[c0 + ns - 1]) + 1):
            s_a = max(int(start_sl[r]), c0)
            s_b = min(int(end_sl[r]), c0 + ns - 1)
            if s_a > s_b:
                continue
            sl = list(range(s_a, s_b + 1))
            tl = [int(base_t[s] + (r - lo_b[s])) for s in sl]
            ents.append(
                (r, sl, tl, start_sl[r] >= c0, end_sl[r] <= c0 + ns - 1)
            )
        chunks.append(
            (c0, ns, int(base_t[c0]),
             int(base_t[c0 + ns] - base_t[c0]), ents)
        )
    assert sum(c[1] for c in chunks) == S_total

    # pack tokens: dense positions
    grp_start = np.zeros(ngroups_keys + 1, dtype=np.int64)
    np.cumsum(cnt, out=grp_start[1:])
    rank = np.arange(len(key_s)) - grp_start[key_s]
    blk_e = key_s % nb
    core_e = key_s // nb
    pos = cum[core_e, blk_e] + rank  # within-core token position

    T = S_total * P
    gidx_np = np.zeros((n_cores, T), dtype=np.int16)
    dstl_np = np.full((n_cores, T), -1.0, dtype=np.float32)
    blk_of = np.full((n_cores, T), -1, dtype=np.int64)
    gidx_np[core_e, pos] = gidx_s.astype(np.int16)
    dstl_np[core_e, pos] = dstl_s.astype(np.float32)
    blk_of[core_e, pos] = blk_e

    # the SWDGE drops TRAILING negative idxs of a gather: ensure the last
    # token position of every chunk has idx >= 0. Swap within the same
    # block's tokens (or with a pad, idx 0, dstl -1).
    for (c0_, ns_, _, _, _) in chunks:
        last = (c0_ + ns_) * P - 1
        for c in range(n_cores):
            if gidx_np[c, last] >= 0:
                continue
            lo_tok = c0_ * P
            seg_ok = (gidx_np[c, lo_tok:last] >= 0) & (
                blk_of[c, lo_tok:last] == blk_of[c, last]
            )
            cand = np.nonzero(seg_ok)[0]
            assert cand.size > 0, "no non-negative idx to swap into chunk tail"
            j = lo_tok + cand[-1]
            for arr in (gidx_np, dstl_np, blk_of):
                arr[c, last], arr[c, j] = arr[c, j], arr[c, last]

    plan = Plan()
    plan.n = n
    plan.n_cores = n_cores
    plan.npc = npc
    plan.nb = nb
    plan.npc_pad = npc_pad
    plan.chunks = chunks
    plan.S_total = S_total
    plan.max_chunk_slots = int(max(c[1] for c in chunks))
    plan.max_chunk_tiles = int(max(c[3] for c in chunks))
    plan.dinv = dinv
    plan.nbA = nbA
    plan.hA = hA
    plan.hB = hB

    # device-ready arrays
    # gather idx: [128, T/16] int16, token j -> (j%16, j//16), replicated x8
    w = gidx_np.reshape(n_cores, -1, 16).transpose(0, 2, 1)  # [c, 16, T/16]
    plan.gidx = np.tile(w, (1, 8, 1)).copy()  # [c, 128, T/16]
    # one-hot H tiles precomputed on host: [128, NT*128] bf16; the tile of
    # token (c, posn) follows the pair/parity mapping. Pads have no bit.
    hmat = np.zeros((n_cores, P, NT * P), dtype=BF16)
    cc, pp = np.nonzero(blk_of >= 0)
    bb = blk_of[cc, pp]
    ss = pp // P
    assert (bb >= lo_b[ss]).all() and (bb <= hi_b[ss]).all()
    t_abs = base_t[ss] + (bb - lo_b[ss])
    dl = dstl_np[cc, pp].astype(np.int64)
    hmat[cc, pp % P, t_abs * P + dl] = 1.0
    plan.hmat = hmat
    plan.NT = NT
    # dinv columns: [128, nb] f32 per core
    dpad = np.zeros((n_cores, npc_pad), dtype=np.float32)
    dpad[:, :npc] = dinv.reshape(n_cores, npc)
    plan.dinv_cols = dpad.reshape(n_cores, nb, P).transpose(0, 2, 1).copy()
    return plan


# ----------------------------------------------------------------------------
# device program
# ----------------------------------------------------------------------------

def build_program(plan, n_layers, d, use_collective=True, use_gather=True,
                  with_bias=True):
    nb = plan.nb
    npc, npc_pad, S_total = plan.npc, plan.npc_pad, plan.S_total
    n, n_cores = plan.n, plan.n_cores
    CS = plan.max_chunk_slots
    CST = plan.max_chunk_tiles
    NT = plan.NT
    L = n_layers
    T16 = S_total * P // 16

    nc = bacc.Bacc("TRN2", target_bir_lowering=False, debug=False,
                   num_devices=n_cores)

    # layer-0 z table, computed on host: full remapped table + own shard
    z1full_in = nc.dram_tensor("z1full", [n, d], BF, kind="ExternalInput")
    z1sh_in = nc.dram_tensor("z1sh", [npc_pad, d], BF, kind="ExternalInput")
    gidx_in = nc.dram_tensor("gidx", [P, T16], I16, kind="ExternalInput")
    hmat_in = nc.dram_tensor("hmat", [P, NT * d], BF, kind="ExternalInput")
    dinv_in = nc.dram_tensor("dinvc", [P, nb], F32, kind="ExternalInput")
    w_in = nc.dram_tensor("wts", [L, d, d], F32, kind="ExternalInput")
    b_in = nc.dram_tensor("brep", [L, P, d], F32, kind="ExternalInput")
    idbf_in = nc.dram_tensor("identbf", [P, P], BF, kind="ExternalInput")
    id_in = nc.dram_tensor("ident", [P, P], F32, kind="ExternalInput")

    out_e = [
        nc.dram_tensor(f"out_e{l + 1}", [npc_pad, d], F32, kind="ExternalOutput")
        for l in range(L)
    ]


    nbA, hA, hB = plan.nbA, plan.hA, plan.hB
    zka = nc.dram_tensor("zka", [hA, d], BF)
    zkb = nc.dram_tensor("zkb", [hB, d], BF)
    zfull = [
        nc.dram_tensor(f"zfull{i}", [n, d], BF, addr_space="Shared")
        for i in range(2)
    ]
    rg = [list(range(n_cores))]

    with tile.TileContext(nc) as tc:
        with (
            tc.tile_pool(name="const", bufs=1) as cpool,
            tc.tile_pool(name="resident", bufs=1) as rpool,
            tc.tile_pool(name="gt", bufs=6) as gpool,
            tc.tile_pool(name="ht", bufs=6) as hpool,
            tc.tile_pool(name="work", bufs=6) as wpool,
            tc.tile_pool(name="pa", bufs=4, space="PSUM") as pa_pool,
            tc.tile_pool(name="pt", bufs=2, space="PSUM") as pt_pool,
            tc.tile_pool(name="pz", bufs=2, space="PSUM") as pz_pool,
        ):
            # ---- constants / resident state ----
            nc.gpsimd.load_library(library_config.mlp)
            identbf_sb = cpool.tile([P, P], BF)
            ident_sb = cpool.tile([P, P], F32)
            dinv_sb = cpool.tile([P, nb], F32)
            gidx_sb = cpool.tile([P, T16], I16)
            w_sb = cpool.tile([P, L * d], F32)
            b_sb = cpool.tile([P, L * d], F32)
            nc.sync.dma_start(identbf_sb[:], idbf_in[:])
            nc.sync.dma_start(ident_sb[:], id_in[:])
            nc.sync.dma_start(dinv_sb[:], dinv_in[:])
            nc.sync.dma_start(gidx_sb[:], gidx_in[:])
            for l in range(L):
                nc.sync.dma_start(w_sb[:, l * d : (l + 1) * d], w_in[l, :, :])
                nc.sync.dma_start(b_sb[:, l * d : (l + 1) * d], b_in[l, :, :])

            x_state_a = rpool.tile([P, nb * d], F32, tag="x_stateA")
            x_state_b = rpool.tile([P, nb * d], F32, tag="x_stateB")
            x_ab = [x_state_a, x_state_b]

            # bf16 z shards, double-buffered by layer parity: the raw z rows
            # feed both the DRAM table writes and the self-loop term (added
            # into pacc via an identity matmul).
            zres_a = rpool.tile([P, nb * d], BF, tag="zresA")
            zres_b = rpool.tile([P, nb * d], BF, tag="zresB")
            zres = [zres_a, zres_b]

            def z_block(l, x_src, r):
                """z row-block r for layer l: z = (dinv*x)@W_l -> zka/zkb,
                bf16 copy kept in zres[l % 2] for the self-loop matmul."""
                xs = wpool.tile([P, d], F32, tag="xs", name=f"xs{l}_{r}")
                nc.scalar.activation(
                    xs[:],
                    x_src[:, r * d : (r + 1) * d],
                    mybir.ActivationFunctionType.Copy,
                    scale=dinv_sb[:, r : r + 1],
                )
                ptr = pt_pool.tile(
                    [P, P], F32, space="PSUM", tag="ptr", name=f"ptr{l}_{r}"
                )
                nc.tensor.transpose(out=ptr[:], in_=xs[:], identity=ident_sb[:])
                xT = wpool.tile([P, P], F32, tag="xT", name=f"xT{l}_{r}")
                nc.scalar.activation(
                    xT[:], ptr[:], mybir.ActivationFunctionType.Copy
                )
                pz = pz_pool.tile(
                    [P, d], F32, space="PSUM", tag="pz", name=f"pz{l}_{r}"
                )
                nc.tensor.matmul(
                    out=pz[:],
                    lhsT=xT[:],
                    rhs=w_sb[:, l * d : (l + 1) * d],
                    start=True,
                    stop=True,
                )
                zcol = zres[l % 2][:, r * d : (r + 1) * d]
                nc.scalar.activation(
                    zcol, pz[:], mybir.ActivationFunctionType.Copy
                )
                if r < nbA:
                    nc.sync.dma_start(
                        zka[r * P : (r + 1) * P, :], zcol
                    )
                else:
                    rows = min(P, npc - r * P)
                    rb = (r - nbA) * P
                    nc.sync.dma_start(
                        zkb[rb : rb + rows, :],
                        zres[l % 2][:rows, r * d : (r + 1) * d],
                    )

            def ag_a(zf_dst):
                if use_collective:
                    nc.gpsimd.collective_compute(
                        "AllGather",
                        mybir.AluOpType.bypass,
                        ins=[zka[:, :]],
                        outs=[zf_dst[: n_cores * hA, :]],
                        replica_groups=rg,
                    )
                else:
                    nc.sync.dma_start(zf_dst[:hA, :], zka[:, :])

            def ag_b(zf_dst):
                if use_collective:
                    nc.gpsimd.collective_compute(
                        "AllGather",
                        mybir.AluOpType.bypass,
                        ins=[zkb[:, :]],
                        outs=[zf_dst[n_cores * hA :, :]],
                        replica_groups=rg,
                    )
                else:
                    nc.sync.dma_start(zf_dst[hA:npc, :], zkb[:, :])

            # prologue: layer-0 z table comes precomputed from the host;
            # only the own bf16 shard (self-loop term) is loaded here.
            for r in range(nb):
                nc.sync.dma_start(
                    zres[0][:, r * d : (r + 1) * d], z1sh_in[r * P : (r + 1) * P, :]
                )

            for l in range(L):
                zf = z1full_in if l == 0 else zfull[(l - 1) % 2]
                zf_next = zfull[l % 2]
                x_next = x_ab[(l + 1) % 2]
                blocks_done = 0
                open_pacc = {}
                # ---- edge phase ----
                # gather chunks + H chunks, then PE aggregation per block;
                # the single gather view is based at row LOW_LIM with signed
                # idx = node - LOW_LIM.
                for ci, (c0, ns, ht0, ntl, ents) in enumerate(plan.chunks):
                    gt = gpool.tile([P, CS, d], BF, tag="gt")
                    if use_gather:
                        nc.gpsimd.dma_gather(
                            out_ap=gt[:, :ns, :],
                            in_ap=zf[LOW_LIM:, :],
                            idxs_ap=gidx_sb[:, c0 * 8 : (c0 + ns) * 8],
                            num_idxs=ns * P,
                            num_idxs_reg=ns * P,
                            elem_size=d,
                            single_packet=False,
                        )
                    else:
                        nc.sync.dma_start(
                            gt[:, :ns, :],
                            zf[: ns * P, :].rearrange(
                                "(s p) c -> p s c", p=P
                            ),
                        )
                    ht = hpool.tile([P, CST * d], BF, tag="ht")
                    nc.sync.dma_start(
                        ht[:, : ntl * d], hmat_in[:, ht0 * d : (ht0 + ntl) * d]
                    )
                    for (r, slots, tiles, fst, lst) in ents:
                            if fst:
                                pacc = pa_pool.tile([P, d], F32, space="PSUM")
                                open_pacc[r] = pacc
                            else:
                                pacc = open_pacc[r]
                            for wi, (s_abs, t_abs) in enumerate(
                                zip(slots, tiles)
                            ):
                                so = s_abs - c0
                                to = t_abs - ht0
                                nc.tensor.matmul(
                                    out=pacc[:],
                                    lhsT=ht[:, to * d : (to + 1) * d],
                                    rhs=gt[:, so, :],
                                    start=(fst and wi == 0),
                                    stop=False,
                                )
                            if not lst:
                                continue
                            del open_pacc[r]
                            # self-loop: pacc += I @ z_own (raw z, bf16)
                            nc.tensor.matmul(
                                out=pacc[:],
                                lhsT=identbf_sb[:],
                                rhs=zres[l % 2][:, r * d : (r + 1) * d],
                                start=False,
                                stop=True,
                            )
                            # ---- epilogue for block r ----
                            # e = dinv*(pacc + z_own) + b, written in place
                            # into the next-layer x state
                            ecol = x_next[:, r * d : (r + 1) * d]
                            nc.scalar.activation(
                                ecol,
                                pacc[:],
                                mybir.ActivationFunctionType.Copy,
                                scale=dinv_sb[:, r : r + 1],
                            )
                            if with_bias:
                                nc.vector.tensor_tensor(
                                    out=ecol,
                                    in0=ecol,
                                    in1=b_sb[:, l * d : (l + 1) * d],
                                    op=mybir.AluOpType.add,
                                )
                            nc.sync.dma_start(
                                out_e[l][r * P : (r + 1) * P, :], ecol
                            )
                            # (total = x0 + e1 + e2 + e3 is summed on the
                            # host from the e outputs — no on-device work)
                            # next layer's z for this block, right behind the
                            # epilogue
                            if l < L - 1:
                                z_block(l + 1, x_next, r)
                                blocks_done += 1
                                # fire the big prefix AllGather as soon as
                                # blocks 0..nbA-1 are through — it overlaps
                                # the remaining ~10% of this layer's gathers
                                if blocks_done == nbA:
                                    ag_a(zf_next)
                if l < L - 1:
                    ag_b(zf_next)
    nc.compile()
    return nc


# ----------------------------------------------------------------------------
# top-level entry
# ----------------------------------------------------------------------------

def make_in_maps(plan, item_emb, weights, biases, n_layers, d):
    n, n_cores, npc, npc_pad = plan.n, plan.n_cores, plan.npc, plan.npc_pad
    x0 = np.asarray(item_emb, dtype=np.float32)[-n:]
    ident_np = np.eye(P, dtype=np.float32)
    identbf_np = np.eye(P, dtype=np.float32).astype(BF16)
    w_np = np.asarray(weights, dtype=np.float32)
    b_np = np.asarray(biases, dtype=np.float32)
    b_rep = np.tile(b_np[:, None, :], (1, P, 1)).astype(np.float32)

    # layer-0 z table on host: z1 = (dinv * x0) @ W0, bf16, remapped to the
    # device table order ([all cores' prefix | all cores' suffix]).
    z1 = ((plan.dinv[:, None] * x0) @ w_np[0]).astype(BF16)
    nodes = np.arange(n, dtype=np.int64)
    s_core = nodes // npc
    s_loc = nodes % npc
    hA, hB = plan.hA, plan.hB
    tpos = np.where(
        s_loc < hA,
        s_core * hA + s_loc,
        n_cores * hA + s_core * hB + (s_loc - hA),
    )
    z1tab = np.zeros((n, d), dtype=BF16)
    z1tab[tpos] = z1

    in_maps = []
    for c in range(n_cores):

        z1sh = np.zeros((npc_pad, d), dtype=BF16)
        z1sh[:npc] = z1[c * npc : (c + 1) * npc]
        in_maps.append(
            {
                "z1full": z1tab,
                "z1sh": z1sh,
                "gidx": plan.gidx[c],
                "hmat": plan.hmat[c],
                "dinvc": plan.dinv_cols[c],
                "wts": w_np,
                "brep": b_rep,
                "identbf": identbf_np,
                "ident": ident_np,
            }
        )
    return in_maps


def assemble_outputs(plan, results, item_emb, n_layers):
    n, n_cores, npc = plan.n, plan.n_cores, plan.npc
    x0 = np.asarray(item_emb, dtype=np.float32)[-n:]
    es = [
        np.concatenate(
            [results[c][f"out_e{l + 1}"][:npc] for c in range(n_cores)]
        )
        for l in range(n_layers)
    ]
    tot = x0.copy()
    for e in es:
        tot += e
    return (tot, x0, *es)


_CACHE = {}


def kernel(item_emb, weights, biases, edge_index, item_nums):
    from concourse.bass_utils import run_bass_kernel_spmd

    n = int(item_nums)
    L, d, _ = np.asarray(weights).shape
    n_cores = 8

    plan = build_plan(np.asarray(edge_index), n, n_cores)
    nc = build_program(plan, L, d, with_bias=bool(np.any(np.asarray(biases))))
    in_maps = make_in_maps(plan, item_emb, weights, biases, L, d)
    res = run_bass_kernel_spmd(nc, in_maps, list(range(n_cores)))
    return assemble_outputs(plan, res.results, item_emb, L)



# revision 71
# speedup vs baseline: 1.0595x; 1.0595x over previous
"""3-layer GCN encoder (PyG GCNConv semantics) on 8 Trainium2 NeuronCores.

Strategy (dst-sharded message passing, gather-descriptor-rate bound):
  - Nodes are 1D-partitioned across the 8 cores (node n -> core n // (N/8)).
  - Per layer l:  z = (dinv * x) @ W_l   computed shard-wise in fp32 on PE,
    rounded to bf16, AllGathered into a full [N, D] bf16 node-order table in
    DRAM. The layer-0 table is precomputed on the HOST (plain numpy), so the
    device starts gathering immediately. (norm factorizes: out[i] = dinv[i]
    * sum_e dinv[src_e] * (x@W)[src_e], so no per-edge scaling is needed.)
  - Each core owns the edges whose dst lands in its shard. Edge messages are
    fetched with GPSIMD dma_gather (HBM->SBUF, 256B rows) through a SINGLE
    base view at row 32768 with SIGNED int16 idx = node - 32768 (the SWDGE
    address mul-acc is unsigned*signed; verified on HW). Only trailing
    negative idxs of a chunk are dropped, so the host keeps the last token
    of every chunk non-negative.
  - Aggregation on PE: per 128-token slot, a HOST-precomputed one-hot tile
    H[t, dstlocal_t] (bf16, streamed per chunk via HWDGE) is matmul'ed with
    the gathered tokens, accumulating fp32 PSUM per 128-node dst block; the
    self-loop term is folded in as one extra `identity @ z_own` matmul.
    Blocks are packed in PAIRS sharing a slot range (halves slot-padding);
    the per-(slot,block) tile lists are core-invariant (union over cores).
  - Epilogue: e = dinv * (agg + z_own) via ONE ScalarE activation (per-
    partition scale); nothing on the Vector engine touches the critical
    path (DVE ops crawl 25-150x while gathers run; ACT is immune).

Host side: edge sort by (core, block), pair-dense packing with (idx=0,
dstl=-1) pad dummies (H row all-zero), H/idx array packing, layer-0 table
compute, and final unshard (concat + slice).
"""

import math

import numpy as np
import ml_dtypes

from concourse import bass, bacc, mybir, library_config
import concourse.tile as tile

BF16 = ml_dtypes.bfloat16
P = 128
LOW_LIM = 32768
PBASE = 16384  # base view row for prefix-section gathers
F32 = mybir.dt.float32
BF = mybir.dt.bfloat16
I16 = mybir.dt.int16


# ----------------------------------------------------------------------------
# host-side preprocessing
# ----------------------------------------------------------------------------

class Plan:
    """Static (core-invariant) program structure + per-core packed arrays."""
    pass


def build_plan(edge_index, n, n_cores, group_blocks=3, max_chunk_slots=28):
    src_e = np.asarray(edge_index[0], dtype=np.int64)
    dst_e = np.asarray(edge_index[1], dtype=np.int64)
    # self-loops are handled analytically on-device (identity matmul), so the
    # token stream only carries the real edges; degree still counts them.
    src = src_e
    dst = dst_e

    deg = (np.bincount(dst, minlength=n) + 1).astype(np.float64)
    dinv = (1.0 / np.sqrt(deg)).astype(np.float32)

    assert n % n_cores == 0
    npc = n // n_cores
    nb = math.ceil(npc / P)
    npc_pad = nb * P

    # z-table is in plain node order; gathers use ONE base view at row
    # LOW_LIM with SIGNED int16 indices idx = node - LOW_LIM (the SWDGE
    # mul-acc is unsigned*signed, so negative idx reads below the view).
    assert n - LOW_LIM <= LOW_LIM

    # table order: [all cores' prefix blocks 0..nbA-1 | all cores' suffix].
    # Tokens are split into two SECTIONS by which AllGather writes their
    # source rows: section P (prefix refs, gathered first — depends only on
    # AG_A) and section S (suffix refs, gathered last — depends on AG_B).
    # AG_A fires mid-S-sweep of the producing layer and AG_B right at its
    # end; both hide under gather streams, so layer boundaries cost ~0.
    nbA = 28
    hA = nbA * P
    hB = npc - hA
    s_core = src // npc
    s_loc = src % npc
    tpos = np.where(
        s_loc < hA,
        s_core * hA + s_loc,
        n_cores * hA + s_core * hB + (s_loc - hA),
    )
    sec = (tpos >= n_cores * hA).astype(np.int64)
    # per-section signed-idx base views (both int16-reachable)
    gidx_val = np.where(sec == 0, tpos - PBASE, tpos - LOW_LIM)

    core = dst // npc
    blk = (dst % npc) // P
    dstl = (dst % npc) % P

    # stable-sort edges by (core, sec, blk)
    key = (core * 2 + sec) * nb + blk
    order = np.argsort(key, kind="stable")
    key_s = key[order]
    gidx_s = gidx_val[order]
    dstl_s = dstl[order]

    ngroups_keys = n_cores * 2 * nb
    cnt = np.bincount(key_s, minlength=ngroups_keys)
    cnt_sb = cnt.reshape(n_cores, 2, nb)
    assert (cnt_sb > 0).all()

    # FULL-DENSE packing per section; section S starts at slot SP.
    cum = np.zeros((n_cores, 2, nb + 1), dtype=np.int64)
    np.cumsum(cnt_sb, axis=2, out=cum[:, :, 1:])
    SP = int(math.ceil(cum[:, 0, nb].max() / P))
    SS = int(math.ceil(cum[:, 1, nb].max() / P))
    S_total = SP + SS
    sec_off = np.array([0, SP], dtype=np.int64)
    # per-(section, block) program-level slot coverage (union over cores)
    start2 = np.zeros((2, nb), dtype=np.int64)
    end2 = np.zeros((2, nb), dtype=np.int64)
    for q in (0, 1):
        start2[q] = (cum[:, q, :nb] // P).min(axis=0) + sec_off[q]
        e = ((cum[:, q, :nb] + cnt_sb[:, q] - 1) // P).max(axis=0) + sec_off[q]
        end2[q] = np.maximum(e, start2[q])
        assert (np.diff(start2[q]) >= 0).all() and (np.diff(end2[q]) >= 0).all()
    assert end2[0].max() < SP
    # blocks covering slot s: contiguous [lo_b(s), hi_b(s)] within a section
    lo_b = np.zeros(S_total, dtype=np.int64)
    hi_b = np.zeros(S_total, dtype=np.int64)
    for q, (a, b) in enumerate(((0, SP), (SP, S_total))):
        sa = np.arange(a, b)
        lo_b[a:b] = np.searchsorted(end2[q], sa, side="left")
        hi_b[a:b] = np.minimum(
            np.searchsorted(start2[q], sa, side="right") - 1, nb - 1
        )
    assert (lo_b <= hi_b).all()
    ntile_s = hi_b - lo_b + 1
    base_t = np.zeros(S_total + 1, dtype=np.int64)
    np.cumsum(ntile_s, out=base_t[1:])
    NT = int(base_t[S_total])

    # chunk boundaries per section; the FINAL chunks of the kernel-ending
    # S-section are kept small so the post-gather agg tail is short
    bounds = []
    for (a, b, small_tail) in ((0, SP, False), (SP, S_total, True)):
        c0 = a
        lim = b - (min(24, b - a) if small_tail else 0)
        while c0 < lim:
            ns = min(max_chunk_slots, lim - c0)
            bounds.append((c0, ns))
            c0 += ns
        while c0 < b:
            ns = min(8, b - c0)
            bounds.append((c0, ns))
            c0 += ns

    chunks = []  # (slot0, ns, tile0, ntiles, [(r, slots, tiles, fst, lst)])
    for (c0, ns) in bounds:
        q = 0 if c0 < SP else 1
        start_sl, end_sl = start2[q], end2[q]
        ents = []
        for r in range(int(lo_b[c0]), int(hi_b[c0 + ns - 1]) + 1):
            s_a = max(int(start_sl[r]), c0)
            s_b = min(int(end_sl[r]), c0 + ns - 1)
            if s_a > s_b:
                continue
            sl = list(range(s_a, s_b + 1))
            tl = [int(base_t[s] + (r - lo_b[s])) for s in sl]
            ents.append(
                (r, sl, tl, start_sl[r] >= c0, end_sl[r] <= c0 + ns - 1)
            )
        chunks.append(
            (c0, ns, int(base_t[c0]),
             int(base_t[c0 + ns] - base_t[c0]), ents)
        )
    assert sum(c[1] for c in chunks) == S_total

    # pack tokens: dense positions per (core, section)
    grp_start = np.zeros(ngroups_keys + 1, dtype=np.int64)
    np.cumsum(cnt, out=grp_start[1:])
    rank = np.arange(len(key_s)) - grp_start[key_s]
    blk_e = key_s % nb
    sec_e = (key_s // nb) % 2
    core_e = key_s // (2 * nb)
    pos = (
        sec_off[sec_e] * P + cum[core_e, sec_e, blk_e] + rank
    )  # within-core token position

    T = S_total * P
    gidx_np = np.zeros((n_cores, T), dtype=np.int16)
    dstl_np = np.full((n_cores, T), -1.0, dtype=np.float32)
    blk_of = np.full((n_cores, T), -1, dtype=np.int64)
    gidx_np[core_e, pos] = gidx_s.astype(np.int16)
    dstl_np[core_e, pos] = dstl_s.astype(np.float32)
    blk_of[core_e, pos] = blk_e

    # the SWDGE drops TRAILING negative idxs of a gather: ensure the last
    # token position of every chunk has idx >= 0. Swap within the same
    # block's tokens (or with a pad, idx 0, dstl -1).
    for (c0_, ns_, _, _, _) in chunks:
        last = (c0_ + ns_) * P - 1
        for c in range(n_cores):
            if gidx_np[c, last] >= 0:
                continue
            lo_tok = c0_ * P
            seg_ok = (gidx_np[c, lo_tok:last] >= 0) & (
                blk_of[c, lo_tok:last] == blk_of[c, last]
            )
            cand = np.nonzero(seg_ok)[0]
            assert cand.size > 0, "no non-negative idx to swap into chunk tail"
            j = lo_tok + cand[-1]
            for arr in (gidx_np, dstl_np, blk_of):
                arr[c, last], arr[c, j] = arr[c, j], arr[c, last]

    plan = Plan()
    plan.n = n
    plan.n_cores = n_cores
    plan.npc = npc
    plan.nb = nb
    plan.npc_pad = npc_pad
    plan.chunks = chunks
    plan.S_total = S_total
    plan.max_chunk_slots = int(max(c[1] for c in chunks))
    plan.max_chunk_tiles = int(max(c[3] for c in chunks))
    plan.dinv = dinv
    plan.nbA = nbA
    plan.hA = hA
    plan.SP = SP
    plan.hB = hB

    # device-ready arrays
    # gather idx: [128, T/16] int16, token j -> (j%16, j//16), replicated x8
    w = gidx_np.reshape(n_cores, -1, 16).transpose(0, 2, 1)  # [c, 16, T/16]
    plan.gidx = np.tile(w, (1, 8, 1)).copy()  # [c, 128, T/16]
    # one-hot H tiles precomputed on host: [128, NT*128] bf16; the tile of
    # token (c, posn) follows the pair/parity mapping. Pads have no bit.
    hmat = np.zeros((n_cores, P, NT * P), dtype=BF16)
    cc, pp = np.nonzero(blk_of >= 0)
    bb = blk_of[cc, pp]
    ss = pp // P
    assert (bb >= lo_b[ss]).all() and (bb <= hi_b[ss]).all()
    t_abs = base_t[ss] + (bb - lo_b[ss])
    dl = dstl_np[cc, pp].astype(np.int64)
    hmat[cc, pp % P, t_abs * P + dl] = 1.0
    plan.hmat = hmat
    plan.NT = NT
    # dinv columns: [128, nb] f32 per core
    dpad = np.zeros((n_cores, npc_pad), dtype=np.float32)
    dpad[:, :npc] = dinv.reshape(n_cores, npc)
    plan.dinv_cols = dpad.reshape(n_cores, nb, P).transpose(0, 2, 1).copy()
    return plan


# ----------------------------------------------------------------------------
# device program
# ----------------------------------------------------------------------------

def build_program(plan, n_layers, d, use_collective=True, use_gather=True,
                  with_bias=True):
    nb = plan.nb
    npc, npc_pad, S_total = plan.npc, plan.npc_pad, plan.S_total
    n, n_cores = plan.n, plan.n_cores
    CS = plan.max_chunk_slots
    CST = plan.max_chunk_tiles
    NT = plan.NT
    L = n_layers
    T16 = S_total * P // 16

    nc = bacc.Bacc("TRN2", target_bir_lowering=False, debug=False,
                   num_devices=n_cores)

    # layer-0 z table, computed on host: full remapped table + own shard
    z1full_in = nc.dram_tensor("z1full", [n, d], BF, kind="ExternalInput")
    z1sh_in = nc.dram_tensor("z1sh", [npc_pad, d], BF, kind="ExternalInput")
    gidx_in = nc.dram_tensor("gidx", [P, T16], I16, kind="ExternalInput")
    hmat_in = nc.dram_tensor("hmat", [P, NT * d], BF, kind="ExternalInput")
    dinv_in = nc.dram_tensor("dinvc", [P, nb], F32, kind="ExternalInput")
    w_in = nc.dram_tensor("wts", [L, d, d], F32, kind="ExternalInput")
    b_in = nc.dram_tensor("brep", [L, P, d], F32, kind="ExternalInput")
    idbf_in = nc.dram_tensor("identbf", [P, P], BF, kind="ExternalInput")
    id_in = nc.dram_tensor("ident", [P, P], F32, kind="ExternalInput")

    out_e = [
        nc.dram_tensor(f"out_e{l + 1}", [npc_pad, d], F32, kind="ExternalOutput")
        for l in range(L)
    ]


    nbA, hA, hB = plan.nbA, plan.hA, plan.hB
    zka = nc.dram_tensor("zka", [hA, d], BF)
    zkb = nc.dram_tensor("zkb", [hB, d], BF)
    zfull = [
        nc.dram_tensor(f"zfull{i}", [n, d], BF, addr_space="Shared")
        for i in range(2)
    ]
    rg = [list(range(n_cores))]

    with tile.TileContext(nc) as tc:
        with (
            tc.tile_pool(name="const", bufs=1) as cpool,
            tc.tile_pool(name="resident", bufs=1) as rpool,
            tc.tile_pool(name="gt", bufs=5) as gpool,
            tc.tile_pool(name="ht", bufs=4) as hpool,
            tc.tile_pool(name="work", bufs=6) as wpool,
            tc.tile_pool(name="pa", bufs=4, space="PSUM") as pa_pool,
            tc.tile_pool(name="pt", bufs=2, space="PSUM") as pt_pool,
            tc.tile_pool(name="pz", bufs=2, space="PSUM") as pz_pool,
        ):
            # ---- constants / resident state ----
            nc.gpsimd.load_library(library_config.mlp)
            identbf_sb = cpool.tile([P, P], BF)
            ident_sb = cpool.tile([P, P], F32)
            dinv_sb = cpool.tile([P, nb], F32)
            gidx_sb = cpool.tile([P, T16], I16)
            w_sb = cpool.tile([P, L * d], F32)
            b_sb = cpool.tile([P, L * d], F32)
            nc.sync.dma_start(identbf_sb[:], idbf_in[:])
            nc.sync.dma_start(ident_sb[:], id_in[:])
            nc.sync.dma_start(dinv_sb[:], dinv_in[:])
            nc.sync.dma_start(gidx_sb[:], gidx_in[:])
            for l in range(L):
                nc.sync.dma_start(w_sb[:, l * d : (l + 1) * d], w_in[l, :, :])
                nc.sync.dma_start(b_sb[:, l * d : (l + 1) * d], b_in[l, :, :])

            x_state_a = rpool.tile([P, nb * d], F32, tag="x_stateA")
            x_state_b = rpool.tile([P, nb * d], F32, tag="x_stateB")
            x_ab = [x_state_a, x_state_b]

            # bf16 z shards, double-buffered by layer parity: the raw z rows
            # feed both the DRAM table writes and the self-loop term (added
            # into pacc via an identity matmul).
            zres_a = rpool.tile([P, nb * d], BF, tag="zresA")
            zres_b = rpool.tile([P, nb * d], BF, tag="zresB")
            zres = [zres_a, zres_b]
            # bf16 prefix-section partial sums, spilled between sections
            part_sb = rpool.tile([P, nb * d], BF, tag="partial")

            def z_block(l, x_src, r):
                """z row-block r for layer l: z = (dinv*x)@W_l -> zka/zkb,
                bf16 copy kept in zres[l % 2] for the self-loop matmul."""
                xs = wpool.tile([P, d], F32, tag="xs", name=f"xs{l}_{r}")
                nc.scalar.activation(
                    xs[:],
                    x_src[:, r * d : (r + 1) * d],
                    mybir.ActivationFunctionType.Copy,
                    scale=dinv_sb[:, r : r + 1],
                )
                ptr = pt_pool.tile(
                    [P, P], F32, space="PSUM", tag="ptr", name=f"ptr{l}_{r}"
                )
                nc.tensor.transpose(out=ptr[:], in_=xs[:], identity=ident_sb[:])
                xT = wpool.tile([P, P], F32, tag="xT", name=f"xT{l}_{r}")
                nc.scalar.activation(
                    xT[:], ptr[:], mybir.ActivationFunctionType.Copy
                )
                pz = pz_pool.tile(
                    [P, d], F32, space="PSUM", tag="pz", name=f"pz{l}_{r}"
                )
                nc.tensor.matmul(
                    out=pz[:],
                    lhsT=xT[:],
                    rhs=w_sb[:, l * d : (l + 1) * d],
                    start=True,
                    stop=True,
                )
                zcol = zres[l % 2][:, r * d : (r + 1) * d]
                nc.scalar.activation(
                    zcol, pz[:], mybir.ActivationFunctionType.Copy
                )
                if r < nbA:
                    nc.sync.dma_start(
                        zka[r * P : (r + 1) * P, :], zcol
                    )
                else:
                    rows = min(P, npc - r * P)
                    rb = (r - nbA) * P
                    nc.sync.dma_start(
                        zkb[rb : rb + rows, :],
                        zres[l % 2][:rows, r * d : (r + 1) * d],
                    )

            def ag_a(zf_dst):
                if use_collective:
                    nc.gpsimd.collective_compute(
                        "AllGather",
                        mybir.AluOpType.bypass,
                        ins=[zka[:, :]],
                        outs=[zf_dst[: n_cores * hA, :]],
                        replica_groups=rg,
                    )
                else:
                    nc.sync.dma_start(zf_dst[:hA, :], zka[:, :])

            def ag_b(zf_dst):
                if use_collective:
                    nc.gpsimd.collective_compute(
                        "AllGather",
                        mybir.AluOpType.bypass,
                        ins=[zkb[:, :]],
                        outs=[zf_dst[n_cores * hA :, :]],
                        replica_groups=rg,
                    )
                else:
                    nc.sync.dma_start(zf_dst[hA:npc, :], zkb[:, :])

            # prologue: layer-0 z table comes precomputed from the host;
            # only the own bf16 shard (self-loop term) is loaded here.
            for r in range(nb):
                nc.sync.dma_start(
                    zres[0][:, r * d : (r + 1) * d], z1sh_in[r * P : (r + 1) * P, :]
                )

            for l in range(L):
                zf = z1full_in if l == 0 else zfull[(l - 1) % 2]
                zf_next = zfull[l % 2]
                x_next = x_ab[(l + 1) % 2]
                blocks_done = 0
                open_pacc = {}
                # ---- edge phase ----
                # gather chunks + H chunks, then PE aggregation per block;
                # the single gather view is based at row LOW_LIM with signed
                # idx = node - LOW_LIM.
                for ci, (c0, ns, ht0, ntl, ents) in enumerate(plan.chunks):
                    sec = 0 if c0 < plan.SP else 1
                    # section P's view lies inside AG_A's write range, so
                    # P-chunks only wait on AG_A; S-chunks wait on AG_B.
                    src_view = (
                        zf[PBASE : n_cores * hA, :] if sec == 0
                        else zf[LOW_LIM:, :]
                    )
                    gt = gpool.tile([P, CS, d], BF, tag="gt")
                    if use_gather:
                        nc.gpsimd.dma_gather(
                            out_ap=gt[:, :ns, :],
                            in_ap=src_view,
                            idxs_ap=gidx_sb[:, c0 * 8 : (c0 + ns) * 8],
                            num_idxs=ns * P,
                            num_idxs_reg=ns * P,
                            elem_size=d,
                            single_packet=False,
                        )
                    else:
                        nc.sync.dma_start(
                            gt[:, :ns, :],
                            zf[: ns * P, :].rearrange(
                                "(s p) c -> p s c", p=P
                            ),
                        )
                    ht = hpool.tile([P, CST * d], BF, tag="ht")
                    nc.sync.dma_start(
                        ht[:, : ntl * d], hmat_in[:, ht0 * d : (ht0 + ntl) * d]
                    )
                    for (r, slots, tiles, fst, lst) in ents:
                            if fst:
                                pacc = pa_pool.tile([P, d], F32, space="PSUM")
                                open_pacc[r] = pacc
                                if sec == 1:
                                    # re-inject the P-section partial sum
                                    nc.tensor.matmul(
                                        out=pacc[:],
                                        lhsT=identbf_sb[:],
                                        rhs=part_sb[:, r * d : (r + 1) * d],
                                        start=True,
                                        stop=False,
                                    )
                            else:
                                pacc = open_pacc[r]
                            nsl = len(slots)
                            for wi, (s_abs, t_abs) in enumerate(
                                zip(slots, tiles)
                            ):
                                so = s_abs - c0
                                to = t_abs - ht0
                                nc.tensor.matmul(
                                    out=pacc[:],
                                    lhsT=ht[:, to * d : (to + 1) * d],
                                    rhs=gt[:, so, :],
                                    start=(fst and wi == 0 and sec == 0),
                                    stop=(
                                        sec == 0 and lst and wi == nsl - 1
                                    ),
                                )
                            if not lst:
                                continue
                            del open_pacc[r]
                            if sec == 0:
                                # spill the P-section partial (bf16) and
                                # release the PSUM tile
                                nc.scalar.activation(
                                    part_sb[:, r * d : (r + 1) * d],
                                    pacc[:],
                                    mybir.ActivationFunctionType.Copy,
                                )
                                continue
                            # self-loop: pacc += I @ z_own (raw z, bf16)
                            nc.tensor.matmul(
                                out=pacc[:],
                                lhsT=identbf_sb[:],
                                rhs=zres[l % 2][:, r * d : (r + 1) * d],
                                start=False,
                                stop=True,
                            )
                            # ---- epilogue for block r ----
                            # e = dinv*(pacc + z_own) + b, written in place
                            # into the next-layer x state
                            ecol = x_next[:, r * d : (r + 1) * d]
                            nc.scalar.activation(
                                ecol,
                                pacc[:],
                                mybir.ActivationFunctionType.Copy,
                                scale=dinv_sb[:, r : r + 1],
                            )
                            if with_bias:
                                nc.vector.tensor_tensor(
                                    out=ecol,
                                    in0=ecol,
                                    in1=b_sb[:, l * d : (l + 1) * d],
                                    op=mybir.AluOpType.add,
                                )
                            nc.sync.dma_start(
                                out_e[l][r * P : (r + 1) * P, :], ecol
                            )
                            # (total = x0 + e1 + e2 + e3 is summed on the
                            # host from the e outputs — no on-device work)
                            # next layer's z for this block, right behind the
                            # epilogue
                            if l < L - 1:
                                z_block(l + 1, x_next, r)
                                blocks_done += 1
                                # fire the big prefix AllGather as soon as
                                # blocks 0..nbA-1 are through — it overlaps
                                # the remaining ~10% of this layer's gathers
                                if blocks_done == nbA:
                                    ag_a(zf_next)
                if l < L - 1:
                    ag_b(zf_next)
    nc.compile()
    return nc


# ----------------------------------------------------------------------------
# top-level entry
# ----------------------------------------------------------------------------

def make_in_maps(plan, item_emb, weights, biases, n_layers, d):
    n, n_cores, npc, npc_pad = plan.n, plan.n_cores, plan.npc, plan.npc_pad
    x0 = np.asarray(item_emb, dtype=np.float32)[-n:]
    ident_np = np.eye(P, dtype=np.float32)
    identbf_np = np.eye(P, dtype=np.float32).astype(BF16)
    w_np = np.asarray(weights, dtype=np.float32)
    b_np = np.asarray(biases, dtype=np.float32)
    b_rep = np.tile(b_np[:, None, :], (1, P, 1)).astype(np.float32)

    # layer-0 z table on host: z1 = (dinv * x0) @ W0, bf16, remapped to the
    # device table order ([all cores' prefix | all cores' suffix]).
    z1 = ((plan.dinv[:, None] * x0) @ w_np[0]).astype(BF16)
    nodes = np.arange(n, dtype=np.int64)
    s_core = nodes // npc
    s_loc = nodes % npc
    hA, hB = plan.hA, plan.hB
    tpos = np.where(
        s_loc < hA,
        s_core * hA + s_loc,
        n_cores * hA + s_core * hB + (s_loc - hA),
    )
    z1tab = np.zeros((n, d), dtype=BF16)
    z1tab[tpos] = z1

    in_maps = []
    for c in range(n_cores):

        z1sh = np.zeros((npc_pad, d), dtype=BF16)
        z1sh[:npc] = z1[c * npc : (c + 1) * npc]
        in_maps.append(
            {
                "z1full": z1tab,
                "z1sh": z1sh,
                "gidx": plan.gidx[c],
                "hmat": plan.hmat[c],
                "dinvc": plan.dinv_cols[c],
                "wts": w_np,
                "brep": b_rep,
                "identbf": identbf_np,
                "ident": ident_np,
            }
        )
    return in_maps


def assemble_outputs(plan, results, item_emb, n_layers):
    n, n_cores, npc = plan.n, plan.n_cores, plan.npc
    x0 = np.asarray(item_emb, dtype=np.float32)[-n:]
    es = [
        np.concatenate(
            [results[c][f"out_e{l + 1}"][:npc] for c in range(n_cores)]
        )
        for l in range(n_layers)
    ]
    tot = x0.copy()
    for e in es:
        tot += e
    return (tot, x0, *es)


_CACHE = {}


def kernel(item_emb, weights, biases, edge_index, item_nums):
    from concourse.bass_utils import run_bass_kernel_spmd

    n = int(item_nums)
    L, d, _ = np.asarray(weights).shape
    n_cores = 8

    plan = build_plan(np.asarray(edge_index), n, n_cores)
    nc = build_program(plan, L, d, with_bias=bool(np.any(np.asarray(biases))))
    in_maps = make_in_maps(plan, item_emb, weights, biases, L, d)
    res = run_bass_kernel_spmd(nc, in_maps, list(range(n_cores)))
    return assemble_outputs(plan, res.results, item_emb, L)



# revision 72
# speedup vs baseline: 1.0683x; 1.0083x over previous
"""3-layer GCN encoder (PyG GCNConv semantics) on 8 Trainium2 NeuronCores.

Strategy (dst-sharded message passing, gather-descriptor-rate bound):
  - Nodes are 1D-partitioned across the 8 cores (node n -> core n // (N/8)).
  - Per layer l:  z = (dinv * x) @ W_l   computed shard-wise in fp32 on PE,
    rounded to bf16, AllGathered into a full [N, D] bf16 node-order table in
    DRAM. The layer-0 table is precomputed on the HOST (plain numpy), so the
    device starts gathering immediately. (norm factorizes: out[i] = dinv[i]
    * sum_e dinv[src_e] * (x@W)[src_e], so no per-edge scaling is needed.)
  - Each core owns the edges whose dst lands in its shard. Edge messages are
    fetched with GPSIMD dma_gather (HBM->SBUF, 256B rows) through a SINGLE
    base view at row 32768 with SIGNED int16 idx = node - 32768 (the SWDGE
    address mul-acc is unsigned*signed; verified on HW). Only trailing
    negative idxs of a chunk are dropped, so the host keeps the last token
    of every chunk non-negative.
  - Aggregation on PE: per 128-token slot, a HOST-precomputed one-hot tile
    H[t, dstlocal_t] (bf16, streamed per chunk via HWDGE) is matmul'ed with
    the gathered tokens, accumulating fp32 PSUM per 128-node dst block; the
    self-loop term is folded in as one extra `identity @ z_own` matmul.
    Blocks are packed in PAIRS sharing a slot range (halves slot-padding);
    the per-(slot,block) tile lists are core-invariant (union over cores).
  - Epilogue: e = dinv * (agg + z_own) via ONE ScalarE activation (per-
    partition scale); nothing on the Vector engine touches the critical
    path (DVE ops crawl 25-150x while gathers run; ACT is immune).

Host side: edge sort by (core, block), pair-dense packing with (idx=0,
dstl=-1) pad dummies (H row all-zero), H/idx array packing, layer-0 table
compute, and final unshard (concat + slice).
"""

import math

import numpy as np
import ml_dtypes

from concourse import bass, bacc, mybir, library_config
import concourse.tile as tile

BF16 = ml_dtypes.bfloat16
P = 128
LOW_LIM = 32768
PBASE = 16384  # base view row for prefix-section gathers
F32 = mybir.dt.float32
BF = mybir.dt.bfloat16
I16 = mybir.dt.int16


# ----------------------------------------------------------------------------
# host-side preprocessing
# ----------------------------------------------------------------------------

class Plan:
    """Static (core-invariant) program structure + per-core packed arrays."""
    pass


def build_plan(edge_index, n, n_cores, group_blocks=3, max_chunk_slots=28):
    src_e = np.asarray(edge_index[0], dtype=np.int64)
    dst_e = np.asarray(edge_index[1], dtype=np.int64)
    # self-loops are handled analytically on-device (identity matmul), so the
    # token stream only carries the real edges; degree still counts them.
    src = src_e
    dst = dst_e

    deg = (np.bincount(dst, minlength=n) + 1).astype(np.float64)
    dinv = (1.0 / np.sqrt(deg)).astype(np.float32)

    assert n % n_cores == 0
    npc = n // n_cores
    nb = math.ceil(npc / P)
    npc_pad = nb * P

    # z-table is in plain node order; gathers use ONE base view at row
    # LOW_LIM with SIGNED int16 indices idx = node - LOW_LIM (the SWDGE
    # mul-acc is unsigned*signed, so negative idx reads below the view).
    assert n - LOW_LIM <= LOW_LIM

    # table order: [all cores' prefix blocks 0..nbA-1 | all cores' suffix].
    # Tokens are split into two SECTIONS by which AllGather writes their
    # source rows: section P (prefix refs, gathered first — depends only on
    # AG_A) and section S (suffix refs, gathered last — depends on AG_B).
    # AG_A fires mid-S-sweep of the producing layer and AG_B right at its
    # end; both hide under gather streams, so layer boundaries cost ~0.
    nbA = 20
    hA = nbA * P
    hB = npc - hA
    s_core = src // npc
    s_loc = src % npc
    tpos = np.where(
        s_loc < hA,
        s_core * hA + s_loc,
        n_cores * hA + s_core * hB + (s_loc - hA),
    )
    sec = (tpos >= n_cores * hA).astype(np.int64)
    # per-section signed-idx base views (both int16-reachable)
    gidx_val = np.where(sec == 0, tpos - PBASE, tpos - LOW_LIM)

    core = dst // npc
    blk = (dst % npc) // P
    dstl = (dst % npc) % P

    # stable-sort edges by (core, sec, blk)
    key = (core * 2 + sec) * nb + blk
    order = np.argsort(key, kind="stable")
    key_s = key[order]
    gidx_s = gidx_val[order]
    dstl_s = dstl[order]

    ngroups_keys = n_cores * 2 * nb
    cnt = np.bincount(key_s, minlength=ngroups_keys)
    cnt_sb = cnt.reshape(n_cores, 2, nb)
    assert (cnt_sb > 0).all()

    # FULL-DENSE packing per section; section S starts at slot SP.
    cum = np.zeros((n_cores, 2, nb + 1), dtype=np.int64)
    np.cumsum(cnt_sb, axis=2, out=cum[:, :, 1:])
    SP = int(math.ceil(cum[:, 0, nb].max() / P))
    SS = int(math.ceil(cum[:, 1, nb].max() / P))
    S_total = SP + SS
    sec_off = np.array([0, SP], dtype=np.int64)
    # per-(section, block) program-level slot coverage (union over cores)
    start2 = np.zeros((2, nb), dtype=np.int64)
    end2 = np.zeros((2, nb), dtype=np.int64)
    for q in (0, 1):
        start2[q] = (cum[:, q, :nb] // P).min(axis=0) + sec_off[q]
        e = ((cum[:, q, :nb] + cnt_sb[:, q] - 1) // P).max(axis=0) + sec_off[q]
        end2[q] = np.maximum(e, start2[q])
        assert (np.diff(start2[q]) >= 0).all() and (np.diff(end2[q]) >= 0).all()
    assert end2[0].max() < SP
    # blocks covering slot s: contiguous [lo_b(s), hi_b(s)] within a section
    lo_b = np.zeros(S_total, dtype=np.int64)
    hi_b = np.zeros(S_total, dtype=np.int64)
    for q, (a, b) in enumerate(((0, SP), (SP, S_total))):
        sa = np.arange(a, b)
        lo_b[a:b] = np.searchsorted(end2[q], sa, side="left")
        hi_b[a:b] = np.minimum(
            np.searchsorted(start2[q], sa, side="right") - 1, nb - 1
        )
    assert (lo_b <= hi_b).all()
    ntile_s = hi_b - lo_b + 1
    base_t = np.zeros(S_total + 1, dtype=np.int64)
    np.cumsum(ntile_s, out=base_t[1:])
    NT = int(base_t[S_total])

    # chunk boundaries per section; the FINAL chunks of the kernel-ending
    # S-section are kept small so the post-gather agg tail is short
    bounds = []
    for (a, b, small_tail) in ((0, SP, False), (SP, S_total, True)):
        c0 = a
        lim = b - (min(24, b - a) if small_tail else 0)
        while c0 < lim:
            ns = min(max_chunk_slots, lim - c0)
            bounds.append((c0, ns))
            c0 += ns
        while c0 < b:
            ns = min(8, b - c0)
            bounds.append((c0, ns))
            c0 += ns

    chunks = []  # (slot0, ns, tile0, ntiles, [(r, slots, tiles, fst, lst)])
    for (c0, ns) in bounds:
        q = 0 if c0 < SP else 1
        start_sl, end_sl = start2[q], end2[q]
        ents = []
        for r in range(int(lo_b[c0]), int(hi_b[c0 + ns - 1]) + 1):
            s_a = max(int(start_sl[r]), c0)
            s_b = min(int(end_sl[r]), c0 + ns - 1)
            if s_a > s_b:
                continue
            sl = list(range(s_a, s_b + 1))
            tl = [int(base_t[s] + (r - lo_b[s])) for s in sl]
            ents.append(
                (r, sl, tl, start_sl[r] >= c0, end_sl[r] <= c0 + ns - 1)
            )
        chunks.append(
            (c0, ns, int(base_t[c0]),
             int(base_t[c0 + ns] - base_t[c0]), ents)
        )
    assert sum(c[1] for c in chunks) == S_total

    # pack tokens: dense positions per (core, section)
    grp_start = np.zeros(ngroups_keys + 1, dtype=np.int64)
    np.cumsum(cnt, out=grp_start[1:])
    rank = np.arange(len(key_s)) - grp_start[key_s]
    blk_e = key_s % nb
    sec_e = (key_s // nb) % 2
    core_e = key_s // (2 * nb)
    pos = (
        sec_off[sec_e] * P + cum[core_e, sec_e, blk_e] + rank
    )  # within-core token position

    T = S_total * P
    gidx_np = np.zeros((n_cores, T), dtype=np.int16)
    dstl_np = np.full((n_cores, T), -1.0, dtype=np.float32)
    blk_of = np.full((n_cores, T), -1, dtype=np.int64)
    gidx_np[core_e, pos] = gidx_s.astype(np.int16)
    dstl_np[core_e, pos] = dstl_s.astype(np.float32)
    blk_of[core_e, pos] = blk_e

    # the SWDGE drops TRAILING negative idxs of a gather: ensure the last
    # token position of every chunk has idx >= 0. Swap within the same
    # block's tokens (or with a pad, idx 0, dstl -1).
    for (c0_, ns_, _, _, _) in chunks:
        last = (c0_ + ns_) * P - 1
        for c in range(n_cores):
            if gidx_np[c, last] >= 0:
                continue
            lo_tok = c0_ * P
            seg_ok = (gidx_np[c, lo_tok:last] >= 0) & (
                blk_of[c, lo_tok:last] == blk_of[c, last]
            )
            cand = np.nonzero(seg_ok)[0]
            assert cand.size > 0, "no non-negative idx to swap into chunk tail"
            j = lo_tok + cand[-1]
            for arr in (gidx_np, dstl_np, blk_of):
                arr[c, last], arr[c, j] = arr[c, j], arr[c, last]

    plan = Plan()
    plan.n = n
    plan.n_cores = n_cores
    plan.npc = npc
    plan.nb = nb
    plan.npc_pad = npc_pad
    plan.chunks = chunks
    plan.S_total = S_total
    plan.max_chunk_slots = int(max(c[1] for c in chunks))
    plan.max_chunk_tiles = int(max(c[3] for c in chunks))
    plan.dinv = dinv
    plan.nbA = nbA
    plan.hA = hA
    plan.SP = SP
    plan.hB = hB

    # device-ready arrays
    # gather idx: [128, T/16] int16, token j -> (j%16, j//16), replicated x8
    w = gidx_np.reshape(n_cores, -1, 16).transpose(0, 2, 1)  # [c, 16, T/16]
    plan.gidx = np.tile(w, (1, 8, 1)).copy()  # [c, 128, T/16]
    # one-hot H tiles precomputed on host: [128, NT*128] bf16; the tile of
    # token (c, posn) follows the pair/parity mapping. Pads have no bit.
    hmat = np.zeros((n_cores, P, NT * P), dtype=BF16)
    cc, pp = np.nonzero(blk_of >= 0)
    bb = blk_of[cc, pp]
    ss = pp // P
    assert (bb >= lo_b[ss]).all() and (bb <= hi_b[ss]).all()
    t_abs = base_t[ss] + (bb - lo_b[ss])
    dl = dstl_np[cc, pp].astype(np.int64)
    hmat[cc, pp % P, t_abs * P + dl] = 1.0
    plan.hmat = hmat
    plan.NT = NT
    # dinv columns: [128, nb] f32 per core
    dpad = np.zeros((n_cores, npc_pad), dtype=np.float32)
    dpad[:, :npc] = dinv.reshape(n_cores, npc)
    plan.dinv_cols = dpad.reshape(n_cores, nb, P).transpose(0, 2, 1).copy()
    return plan


# ----------------------------------------------------------------------------
# device program
# ----------------------------------------------------------------------------

def build_program(plan, n_layers, d, use_collective=True, use_gather=True,
                  with_bias=True):
    nb = plan.nb
    npc, npc_pad, S_total = plan.npc, plan.npc_pad, plan.S_total
    n, n_cores = plan.n, plan.n_cores
    CS = plan.max_chunk_slots
    CST = plan.max_chunk_tiles
    NT = plan.NT
    L = n_layers
    T16 = S_total * P // 16

    nc = bacc.Bacc("TRN2", target_bir_lowering=False, debug=False,
                   num_devices=n_cores)

    # layer-0 z table, computed on host: full remapped table + own shard
    z1full_in = nc.dram_tensor("z1full", [n, d], BF, kind="ExternalInput")
    z1sh_in = nc.dram_tensor("z1sh", [npc_pad, d], BF, kind="ExternalInput")
    gidx_in = nc.dram_tensor("gidx", [P, T16], I16, kind="ExternalInput")
    hmat_in = nc.dram_tensor("hmat", [P, NT * d], BF, kind="ExternalInput")
    dinv_in = nc.dram_tensor("dinvc", [P, nb], F32, kind="ExternalInput")
    w_in = nc.dram_tensor("wts", [L, d, d], F32, kind="ExternalInput")
    b_in = nc.dram_tensor("brep", [L, P, d], F32, kind="ExternalInput")
    idbf_in = nc.dram_tensor("identbf", [P, P], BF, kind="ExternalInput")
    id_in = nc.dram_tensor("ident", [P, P], F32, kind="ExternalInput")

    out_e = [
        nc.dram_tensor(f"out_e{l + 1}", [npc_pad, d], F32, kind="ExternalOutput")
        for l in range(L)
    ]


    nbA, hA, hB = plan.nbA, plan.hA, plan.hB
    zka = nc.dram_tensor("zka", [hA, d], BF)
    zkb = nc.dram_tensor("zkb", [hB, d], BF)
    zfull = [
        nc.dram_tensor(f"zfull{i}", [n, d], BF, addr_space="Shared")
        for i in range(2)
    ]
    rg = [list(range(n_cores))]

    with tile.TileContext(nc) as tc:
        with (
            tc.tile_pool(name="const", bufs=1) as cpool,
            tc.tile_pool(name="resident", bufs=1) as rpool,
            tc.tile_pool(name="gt", bufs=5) as gpool,
            tc.tile_pool(name="ht", bufs=4) as hpool,
            tc.tile_pool(name="work", bufs=6) as wpool,
            tc.tile_pool(name="pa", bufs=4, space="PSUM") as pa_pool,
            tc.tile_pool(name="pt", bufs=2, space="PSUM") as pt_pool,
            tc.tile_pool(name="pz", bufs=2, space="PSUM") as pz_pool,
        ):
            # ---- constants / resident state ----
            nc.gpsimd.load_library(library_config.mlp)
            identbf_sb = cpool.tile([P, P], BF)
            ident_sb = cpool.tile([P, P], F32)
            dinv_sb = cpool.tile([P, nb], F32)
            gidx_sb = cpool.tile([P, T16], I16)
            w_sb = cpool.tile([P, L * d], F32)
            b_sb = cpool.tile([P, L * d], F32)
            nc.sync.dma_start(identbf_sb[:], idbf_in[:])
            nc.sync.dma_start(ident_sb[:], id_in[:])
            nc.sync.dma_start(dinv_sb[:], dinv_in[:])
            nc.sync.dma_start(gidx_sb[:], gidx_in[:])
            for l in range(L):
                nc.sync.dma_start(w_sb[:, l * d : (l + 1) * d], w_in[l, :, :])
                nc.sync.dma_start(b_sb[:, l * d : (l + 1) * d], b_in[l, :, :])

            x_state_a = rpool.tile([P, nb * d], F32, tag="x_stateA")
            x_state_b = rpool.tile([P, nb * d], F32, tag="x_stateB")
            x_ab = [x_state_a, x_state_b]

            # bf16 z shards, double-buffered by layer parity: the raw z rows
            # feed both the DRAM table writes and the self-loop term (added
            # into pacc via an identity matmul).
            zres_a = rpool.tile([P, nb * d], BF, tag="zresA")
            zres_b = rpool.tile([P, nb * d], BF, tag="zresB")
            zres = [zres_a, zres_b]
            # bf16 prefix-section partial sums, spilled between sections
            part_sb = rpool.tile([P, nb * d], BF, tag="partial")

            def z_block(l, x_src, r):
                """z row-block r for layer l: z = (dinv*x)@W_l -> zka/zkb,
                bf16 copy kept in zres[l % 2] for the self-loop matmul."""
                xs = wpool.tile([P, d], F32, tag="xs", name=f"xs{l}_{r}")
                nc.scalar.activation(
                    xs[:],
                    x_src[:, r * d : (r + 1) * d],
                    mybir.ActivationFunctionType.Copy,
                    scale=dinv_sb[:, r : r + 1],
                )
                ptr = pt_pool.tile(
                    [P, P], F32, space="PSUM", tag="ptr", name=f"ptr{l}_{r}"
                )
                nc.tensor.transpose(out=ptr[:], in_=xs[:], identity=ident_sb[:])
                xT = wpool.tile([P, P], F32, tag="xT", name=f"xT{l}_{r}")
                nc.scalar.activation(
                    xT[:], ptr[:], mybir.ActivationFunctionType.Copy
                )
                pz = pz_pool.tile(
                    [P, d], F32, space="PSUM", tag="pz", name=f"pz{l}_{r}"
                )
                nc.tensor.matmul(
                    out=pz[:],
                    lhsT=xT[:],
                    rhs=w_sb[:, l * d : (l + 1) * d],
                    start=True,
                    stop=True,
                )
                zcol = zres[l % 2][:, r * d : (r + 1) * d]
                nc.scalar.activation(
                    zcol, pz[:], mybir.ActivationFunctionType.Copy
                )
                if r < nbA:
                    nc.sync.dma_start(
                        zka[r * P : (r + 1) * P, :], zcol
                    )
                else:
                    rows = min(P, npc - r * P)
                    rb = (r - nbA) * P
                    nc.sync.dma_start(
                        zkb[rb : rb + rows, :],
                        zres[l % 2][:rows, r * d : (r + 1) * d],
                    )

            def ag_a(zf_dst):
                if use_collective:
                    nc.gpsimd.collective_compute(
                        "AllGather",
                        mybir.AluOpType.bypass,
                        ins=[zka[:, :]],
                        outs=[zf_dst[: n_cores * hA, :]],
                        replica_groups=rg,
                    )
                else:
                    nc.sync.dma_start(zf_dst[:hA, :], zka[:, :])

            def ag_b(zf_dst):
                if use_collective:
                    nc.gpsimd.collective_compute(
                        "AllGather",
                        mybir.AluOpType.bypass,
                        ins=[zkb[:, :]],
                        outs=[zf_dst[n_cores * hA :, :]],
                        replica_groups=rg,
                    )
                else:
                    nc.sync.dma_start(zf_dst[hA:npc, :], zkb[:, :])

            # prologue: layer-0 z table comes precomputed from the host;
            # only the own bf16 shard (self-loop term) is loaded here.
            for r in range(nb):
                nc.sync.dma_start(
                    zres[0][:, r * d : (r + 1) * d], z1sh_in[r * P : (r + 1) * P, :]
                )

            for l in range(L):
                zf = z1full_in if l == 0 else zfull[(l - 1) % 2]
                zf_next = zfull[l % 2]
                x_next = x_ab[(l + 1) % 2]
                blocks_done = 0
                open_pacc = {}
                # ---- edge phase ----
                # gather chunks + H chunks, then PE aggregation per block;
                # the single gather view is based at row LOW_LIM with signed
                # idx = node - LOW_LIM.
                for ci, (c0, ns, ht0, ntl, ents) in enumerate(plan.chunks):
                    sec = 0 if c0 < plan.SP else 1
                    # section P's view lies inside AG_A's write range, so
                    # P-chunks only wait on AG_A; S-chunks wait on AG_B.
                    src_view = (
                        zf[PBASE : n_cores * hA, :] if sec == 0
                        else zf[LOW_LIM:, :]
                    )
                    gt = gpool.tile([P, CS, d], BF, tag="gt")
                    if use_gather:
                        nc.gpsimd.dma_gather(
                            out_ap=gt[:, :ns, :],
                            in_ap=src_view,
                            idxs_ap=gidx_sb[:, c0 * 8 : (c0 + ns) * 8],
                            num_idxs=ns * P,
                            num_idxs_reg=ns * P,
                            elem_size=d,
                            single_packet=False,
                        )
                    else:
                        nc.sync.dma_start(
                            gt[:, :ns, :],
                            zf[: ns * P, :].rearrange(
                                "(s p) c -> p s c", p=P
                            ),
                        )
                    ht = hpool.tile([P, CST * d], BF, tag="ht")
                    nc.sync.dma_start(
                        ht[:, : ntl * d], hmat_in[:, ht0 * d : (ht0 + ntl) * d]
                    )
                    for (r, slots, tiles, fst, lst) in ents:
                            if fst:
                                pacc = pa_pool.tile([P, d], F32, space="PSUM")
                                open_pacc[r] = pacc
                                if sec == 1:
                                    # re-inject the P-section partial sum
                                    nc.tensor.matmul(
                                        out=pacc[:],
                                        lhsT=identbf_sb[:],
                                        rhs=part_sb[:, r * d : (r + 1) * d],
                                        start=True,
                                        stop=False,
                                    )
                            else:
                                pacc = open_pacc[r]
                            nsl = len(slots)
                            for wi, (s_abs, t_abs) in enumerate(
                                zip(slots, tiles)
                            ):
                                so = s_abs - c0
                                to = t_abs - ht0
                                nc.tensor.matmul(
                                    out=pacc[:],
                                    lhsT=ht[:, to * d : (to + 1) * d],
                                    rhs=gt[:, so, :],
                                    start=(fst and wi == 0 and sec == 0),
                                    stop=(
                                        sec == 0 and lst and wi == nsl - 1
                                    ),
                                )
                            if not lst:
                                continue
                            del open_pacc[r]
                            if sec == 0:
                                # spill the P-section partial (bf16) and
                                # release the PSUM tile
                                nc.scalar.activation(
                                    part_sb[:, r * d : (r + 1) * d],
                                    pacc[:],
                                    mybir.ActivationFunctionType.Copy,
                                )
                                continue
                            # self-loop: pacc += I @ z_own (raw z, bf16)
                            nc.tensor.matmul(
                                out=pacc[:],
                                lhsT=identbf_sb[:],
                                rhs=zres[l % 2][:, r * d : (r + 1) * d],
                                start=False,
                                stop=True,
                            )
                            # ---- epilogue for block r ----
                            # e = dinv*(pacc + z_own) + b, written in place
                            # into the next-layer x state
                            ecol = x_next[:, r * d : (r + 1) * d]
                            nc.scalar.activation(
                                ecol,
                                pacc[:],
                                mybir.ActivationFunctionType.Copy,
                                scale=dinv_sb[:, r : r + 1],
                            )
                            if with_bias:
                                nc.vector.tensor_tensor(
                                    out=ecol,
                                    in0=ecol,
                                    in1=b_sb[:, l * d : (l + 1) * d],
                                    op=mybir.AluOpType.add,
                                )
                            nc.sync.dma_start(
                                out_e[l][r * P : (r + 1) * P, :], ecol
                            )
                            # (total = x0 + e1 + e2 + e3 is summed on the
                            # host from the e outputs — no on-device work)
                            # next layer's z for this block, right behind the
                            # epilogue
                            if l < L - 1:
                                z_block(l + 1, x_next, r)
                                blocks_done += 1
                                # fire the big prefix AllGather as soon as
                                # blocks 0..nbA-1 are through — it overlaps
                                # the remaining ~10% of this layer's gathers
                                if blocks_done == nbA:
                                    ag_a(zf_next)
                if l < L - 1:
                    ag_b(zf_next)
    nc.compile()
    return nc


# ----------------------------------------------------------------------------
# top-level entry
# ----------------------------------------------------------------------------

def make_in_maps(plan, item_emb, weights, biases, n_layers, d):
    n, n_cores, npc, npc_pad = plan.n, plan.n_cores, plan.npc, plan.npc_pad
    x0 = np.asarray(item_emb, dtype=np.float32)[-n:]
    ident_np = np.eye(P, dtype=np.float32)
    identbf_np = np.eye(P, dtype=np.float32).astype(BF16)
    w_np = np.asarray(weights, dtype=np.float32)
    b_np = np.asarray(biases, dtype=np.float32)
    b_rep = np.tile(b_np[:, None, :], (1, P, 1)).astype(np.float32)

    # layer-0 z table on host: z1 = (dinv * x0) @ W0, bf16, remapped to the
    # device table order ([all cores' prefix | all cores' suffix]).
    z1 = ((plan.dinv[:, None] * x0) @ w_np[0]).astype(BF16)
    nodes = np.arange(n, dtype=np.int64)
    s_core = nodes // npc
    s_loc = nodes % npc
    hA, hB = plan.hA, plan.hB
    tpos = np.where(
        s_loc < hA,
        s_core * hA + s_loc,
        n_cores * hA + s_core * hB + (s_loc - hA),
    )
    z1tab = np.zeros((n, d), dtype=BF16)
    z1tab[tpos] = z1

    in_maps = []
    for c in range(n_cores):

        z1sh = np.zeros((npc_pad, d), dtype=BF16)
        z1sh[:npc] = z1[c * npc : (c + 1) * npc]
        in_maps.append(
            {
                "z1full": z1tab,
                "z1sh": z1sh,
                "gidx": plan.gidx[c],
                "hmat": plan.hmat[c],
                "dinvc": plan.dinv_cols[c],
                "wts": w_np,
                "brep": b_rep,
                "identbf": identbf_np,
                "ident": ident_np,
            }
        )
    return in_maps


def assemble_outputs(plan, results, item_emb, n_layers):
    n, n_cores, npc = plan.n, plan.n_cores, plan.npc
    x0 = np.asarray(item_emb, dtype=np.float32)[-n:]
    es = [
        np.concatenate(
            [results[c][f"out_e{l + 1}"][:npc] for c in range(n_cores)]
        )
        for l in range(n_layers)
    ]
    tot = x0.copy()
    for e in es:
        tot += e
    return (tot, x0, *es)


_CACHE = {}


def kernel(item_emb, weights, biases, edge_index, item_nums):
    from concourse.bass_utils import run_bass_kernel_spmd

    n = int(item_nums)
    L, d, _ = np.asarray(weights).shape
    n_cores = 8

    plan = build_plan(np.asarray(edge_index), n, n_cores)
    nc = build_program(plan, L, d, with_bias=bool(np.any(np.asarray(biases))))
    in_maps = make_in_maps(plan, item_emb, weights, biases, L, d)
    res = run_bass_kernel_spmd(nc, in_maps, list(range(n_cores)))
    return assemble_outputs(plan, res.results, item_emb, L)

